# revision 12
# baseline (speedup 1.0000x reference)
"""2D Haar DWT (single level) on Trainium2, 8-core data-parallel.

Input  x: (8, 512, 512, 32) fp32 NHWC.
Output (ll, lh, hl, hh): each (8, 256, 256, 32) fp32.

Math: the reference (symmetric pad + valid correlation + odd-index
downsample with 2-tap Haar filters) reduces exactly to a 2x2 block
butterfly.  With A=x[2i,2j], B=x[2i,2j+1], C=x[2i+1,2j], D=x[2i+1,2j+1]:
    ll = 0.5*(A+B+C+D)   lh = 0.5*(A+B-C-D)
    hl = 0.5*(A-B+C-D)   hh = 0.5*(A-B-C+D)
(The symmetric padding never reaches the odd-indexed downsample taps.)

Implementation: raw bass (explicit semaphores; Tile's auto-sync emits
>2 sync waits on some instructions which the ISA cannot encode).

Per core = one batch sample, viewed as [256 row-pairs, 2, 16 W-chunks,
1024].  32 tiles (2 partition blocks x 16 W-chunks).  Tiles alternate
between DVE and GPSIMD (both sustain ~2.1-2.6 cyc/elem fp32 2-input ops,
so the split nearly halves elementwise time).  Pipeline per tile:

  SP   : in-DMA  x-chunk -> xt[slot]            (HWDGE sync ring)
  ENG  : st[0] = x0+x1 ; st[1] = x0-x1          (stage 1, H butterfly)
         o[0:2] = st_even + st_odd  -> [ll, lh] (stage 2, W butterfly)
         o[2:4] = st_even - st_odd  -> [hl, hh]
  ACT  : o *= 0.5 in place; out-DMA o -> out4   (HWDGE scalar ring)

Slot reuse is gated by semaphores: SP waits for stage-1 completion
(engine sem) before refilling xt; engines wait for the out-DMA
completion (dma_out counts 16/DMA in ACT-ring FIFO order) before
rewriting o; every wait is a standalone sequencer wait instruction.
"""

from contextlib import ExitStack

import numpy as np

import concourse.mybir as mybir
from concourse.bass import Bass
from concourse.bass_utils import run_bass_kernel_spmd

N_CORES = 8
H, W, C = 512, 512, 32
RP = H // 2              # 256 row pairs
PBLK = RP // 128         # 2 partition blocks
WCH = 16                 # W chunks per row
WC = W // WCH            # 32 W columns per chunk
FE = WC * C              # 1024 floats per row per chunk
NG = WC // 2             # 16 W-pair groups per chunk
OE = NG * C              # 512 floats per subband per chunk
TILES = PBLK * WCH       # 32

F32 = mybir.dt.float32
ALU = mybir.AluOpType

_CACHE = {}


def _tile_coords(gi):
    pb, wc = divmod(gi, WCH)
    return slice(pb * 128, (pb + 1) * 128), wc


def build_nc(split_engines: bool = True, dve_tiles: int = 16, bufs: int = 3):
    """Build the SPMD Bass program (identical on all 8 cores)."""
    nc = Bass()
    x = nc.declare_dram_parameter("x", [RP, 2, WCH, FE], F32, isOutput=False)
    # subband planes ordered (ll, lh, hl, hh)
    out4 = nc.declare_dram_parameter("out4", [RP, WCH, 4, OE], F32, isOutput=True)

    # spread DVE/GPSIMD tile ownership evenly through the stream
    engs = []
    if split_engines:
        acc = 0
        for _ in range(TILES):
            acc += dve_tiles
            if acc >= TILES:
                acc -= TILES
                engs.append("v")
            else:
                engs.append("g")
    else:
        engs = ["v"] * TILES
    tiles_of = {"v": [], "g": []}
    j_of = []
    for gi, e in enumerate(engs):
        j_of.append(len(tiles_of[e]))
        tiles_of[e].append(gi)

    B = bufs

    with ExitStack() as ctx:
        block = ctx.enter_context(nc.Block())
        # Per-slot DMA-completion semaphores.  A slot's DMAs are strictly
        # serialized by the pipeline (the consumer must finish before the
        # producer refills), so each sem never has two in-flight DMAs and
        # "wait >= 16*k" exactly means "k-th DMA on this slot finished".
        # A single global counting sem would be unsound: with several DMAs
        # in flight, 16 increments can arrive from a mix of them.
        sem_in = {}
        sem_out = {}
        sems = {
            "v": ctx.enter_context(nc.semaphore("sem_v")),
            "g": ctx.enter_context(nc.semaphore("sem_g")),
        }
        sem_act = ctx.enter_context(nc.semaphore("sem_act"))
        bufs_of = {}
        for e in ("v", "g"):
            if not tiles_of[e]:
                continue
            bufs_of[e] = (
                ctx.enter_context(nc.sbuf_tensor(f"xt_{e}", [128, B, 2, FE], F32)),
                ctx.enter_context(nc.sbuf_tensor(f"st_{e}", [128, B, 2, FE], F32)),
                ctx.enter_context(nc.sbuf_tensor(f"o_{e}", [128, B, 4, OE], F32)),
            )
            for b in range(B):
                sem_in[e, b] = ctx.enter_context(nc.semaphore(f"sin_{e}{b}"))
                sem_out[e, b] = ctx.enter_context(nc.semaphore(f"sout_{e}{b}"))

        @block.sync
        def _(sp):
            for gi in range(TILES):
                e = engs[gi]
                j = j_of[gi]
                slot = j % B
                if j >= B:
                    # stage 1 of the tile that last used this xt slot done
                    sp.wait_ge(sems[e], 2 * (j - B) + 1)
                rows, wc = _tile_coords(gi)
                xt = bufs_of[e][0]
                sp.dma_start(
                    out=xt[:, slot, :, :], in_=x[rows, :, wc, :]
                ).then_inc(sem_in[e, slot], 16)

        def compute_prog(eng, e):
            my = tiles_of[e]
            sem = sems[e]
            xt, st, o = bufs_of[e]
            for j, gi in enumerate(my):
                slot = j % B
                eng.wait_ge(sem_in[e, slot], 16 * (j // B + 1))
                x0 = xt[:, slot, 0, :]
                x1 = xt[:, slot, 1, :]
                s_ap = st[:, slot, 0, :]
                t_ap = st[:, slot, 1, :]
                eng.tensor_add(out=s_ap, in0=x0, in1=x1)
                if e == "v":
                    ins1 = eng.tensor_sub(out=t_ap, in0=x0, in1=x1)
                else:
                    # gpsimd has no tensor_sub: x0-x1 == (x1 * -1) + x0
                    ins1 = eng.scalar_tensor_tensor(
                        out=t_ap, in0=x1, scalar=-1.0, in1=x0,
                        op0=ALU.mult, op1=ALU.add,
                    )
                ins1.then_inc(sem, 1)

                if j >= B:
                    # out-DMA of the tile that last used this o slot done
                    eng.wait_ge(sem_out[e, slot], 16 * (j // B))

                stv = st[:, slot, :, :].rearrange(
                    "p k (g i c) -> p k g i c", i=2, c=C
                )
                ov = o[:, slot, :, :].rearrange(
                    "p (j k) (g c) -> p j k g c", j=2, c=C
                )
                eng.tensor_add(
                    out=ov[:, 0], in0=stv[:, :, :, 0, :], in1=stv[:, :, :, 1, :]
                )
                if e == "v":
                    ins2 = eng.tensor_sub(
                        out=ov[:, 1], in0=stv[:, :, :, 0, :], in1=stv[:, :, :, 1, :]
                    )
                else:
                    ins2 = eng.scalar_tensor_tensor(
                        out=ov[:, 1], in0=stv[:, :, :, 1, :], scalar=-1.0,
                        in1=stv[:, :, :, 0, :], op0=ALU.mult, op1=ALU.add,
                    )
                ins2.then_inc(sem, 1)

        if tiles_of["v"]:

            @block.vector
            def _(dve):
                compute_prog(dve, "v")

        if tiles_of["g"]:

            @block.gpsimd
            def _(gp):
                compute_prog(gp, "g")

        @block.scalar
        def _(act):
            for gi in range(TILES):
                e = engs[gi]
                j = j_of[gi]
                slot = j % B
                act.wait_ge(sems[e], 2 * j + 2)
                o = bufs_of[e][2]
                oap = o[:, slot, :, :]
                # DMA triggers are sequencer-executed and would race the
                # in-flight datapath op on the same engine: gate explicitly.
                act.mul(oap, oap, 0.5).then_inc(sem_act, 1)
                act.wait_ge(sem_act, gi + 1)
                rows, wc = _tile_coords(gi)
                act.dma_start(
                    out=out4[rows, wc, :, :], in_=oap
                ).then_inc(sem_out[e, slot], 16)
            # all out-DMAs landed before the kernel-end barrier
            for e in ("v", "g"):
                n = len(tiles_of[e])
                if not n:
                    continue
                for b in range(B):
                    uses = len(range(b, n, B))
                    if uses:
                        act.wait_ge(sem_out[e, b], 16 * uses)

    return nc


def _run(x, split_engines=True, dve_tiles=16, build_kwargs=None, **run_kwargs):
    build_kwargs = build_kwargs or {}
    key = (bool(split_engines), dve_tiles, tuple(sorted(build_kwargs.items())))
    if key not in _CACHE:
        _CACHE[key] = build_nc(split_engines, dve_tiles, **build_kwargs)
    nc = _CACHE[key]

    in_maps = [
        {"x": np.ascontiguousarray(x[i]).reshape(RP, 2, WCH, FE)}
        for i in range(N_CORES)
    ]
    res = run_bass_kernel_spmd(nc, in_maps, list(range(N_CORES)), **run_kwargs)

    ll = np.empty((N_CORES, RP, WCH * NG, C), dtype=np.float32)
    lh = np.empty_like(ll)
    hl = np.empty_like(ll)
    hh = np.empty_like(ll)
    for i in range(N_CORES):
        o4 = res.results[i]["out4"]  # (RP, WCH, 4, OE)
        ll[i] = o4[:, :, 0, :].reshape(RP, WCH * NG, C)
        lh[i] = o4[:, :, 1, :].reshape(RP, WCH * NG, C)
        hl[i] = o4[:, :, 2, :].reshape(RP, WCH * NG, C)
        hh[i] = o4[:, :, 3, :].reshape(RP, WCH * NG, C)
    return (ll, lh, hl, hh), res


def kernel(x):
    x = np.asarray(x)
    assert x.shape == (N_CORES, H, W, C), x.shape
    if x.dtype != np.float32:
        x = x.astype(np.float32)
    outs, _ = _run(x)
    return outs
